# revision 3
# baseline (speedup 1.0000x reference)
"""Trainium2 Bass kernel for nn_DiffusionActionHead (MoE-style category routing).

Strategy (host side, inside kernel()):
  - Group the B=32 batch items by cat_id. Each distinct category's work is
    split into two column-halves (output-dim split of the big matmuls), giving
    uniform "half-unit" slots of ~8.6MB (bf16) weight traffic each.
  - Slots are distributed round-robin over the 8 NeuronCores; every core runs
    the SAME program over NSLOT slots (SPMD), with all routing baked into
    host-gathered per-core input arrays. Dummy padding slots replicate slot 0
    and their outputs are discarded.
  - Per-item sinusoidal timestep embeddings (a function of the int timesteps
    input only) are computed on host; all weight-table FLOPs run on device.
  - All weights/activations are cast to bf16 on host: halves HBM traffic
    (the bottleneck; target_regime=memory) and halves PE cycles/row vs f32r.
    PSUM accumulation stays fp32; partial outputs return as bf16 and are
    summed on host in fp32 (total quantization noise ~5e-3 rel, gate 2e-2).
  - Column-half partial outputs are summed on host during unsharding.

Device program per slot (raw Bass, manual semaphores; bf16 matmuls):
  SE1  hT = relu(seW1h^T @ state + b1h)        (4x [128,4] matmuls)
  SE2  sf = hT^T @ seW2h + seb2(half0)         (partial state_feat, 3 o-tiles)
  AE1  aT = (W1 chunks)^T @ actionsT + b1      (12x [128,128], transposed out)
  TT   tt = tauT^T @ W2bh + b2h                (per-item tau contribution)
  X2   x2 = aT^T @ W2ah + broadcast(tt); swish (2 o-tiles of 384)
  TR   x2T chunks via PE transpose
  AE3  out = x2T^T @ W3h + b3(half0)           (partial, 3 o-tiles of 512)

Weight chunks stream through a ring of SBUF buffers; input DMAs ride the SP
HWDGE queue (sem s_dma), output DMAs ride the ACT HWDGE queue (sem s_dmo) so
the SP stream never blocks on compute completion.
"""
import sys

sys.path.insert(0, "/opt/trn_rl_repo")

import contextlib
import numpy as np
import ml_dtypes

import concourse.bass as bass
import concourse.mybir as mybir
from concourse.bass_utils import run_bass_kernel_spmd

F32 = mybir.dt.float32
BF16 = mybir.dt.bfloat16
NPBF16 = ml_dtypes.bfloat16
AF = mybir.ActivationFunctionType

E, STATE_DIM, ACT_DIM, HID, EMB = 32, 64, 32, 1024, 1536
B, T = 32, 32
N_CORES = 8
ITEMS_PER_SLOT = 4          # token tile = 4*32 = 128 tokens
HH = HID // 2               # 512: h-column half for the state encoder
OH = EMB // 2               # 768: output-column half for the action encoder
RS = 6                      # SP-queue ring slots of [128, 6144] bf16
RA = 4                      # ACT-queue ring slots of [128, 4608] bf16

# PIN layout columns
PIN_TAU = 0        # [128, 48]  tauT chunks (12 k-chunks x 4 items)
PIN_B1C = 48       # [128, 12]  ae_b1 per-partition chunks
PIN_SB1 = 60       # [128, 4]   se_b1 half, per-partition chunks
PIN_ACT = 64       # [0:32, 128] actionsT
PIN_ST = 192       # [0:64, 4]  stateT
PIN_W = 196

# BIAS row layout (free dim)
BIA_B2 = 0         # [768]  ae_b2[O]
BIA_B3 = 768       # [1536] ae_b3 (half0 only)
BIA_SB2 = 2304     # [1536] se_b2 (half0 only)
BIA_W = 3840


def _sinusoid(ts):
    half = EMB // 2
    div = np.exp(-np.log(np.float32(10000.0)) * np.arange(half, dtype=np.float32) / np.float32(half))
    ang = ts.astype(np.float32)[:, None] * div[None, :]
    return np.concatenate([np.sin(ang), np.cos(ang)], axis=1).astype(np.float32)


# ---------------------------------------------------------------------------
# Build-time plan. Ops live in three engine streams: "dma" (SP: input DMAs),
# "pe" (matmuls/transposes), "actq" (ACT: activations AND output DMAs), "dve".
# Sem protocol: every SP DMA incs s_dma by 16; every ACT-queue DMA incs s_dmo
# by 16; every PE op incs s_pe by 1; every activation incs s_act by 1; every
# DVE copy incs s_dve by 1. Cross-engine deps become standalone wait_ge ops.
# ---------------------------------------------------------------------------
class _Buf:
    __slots__ = ("writer", "readers")

    def __init__(self):
        self.writer = None      # (sem, value, stream)
        self.readers = []


class _Plan:
    def __init__(self):
        self.dma = []
        self.pe = []
        self.actq = []
        self.dve = []
        self.counts = {}

    def emit(self, stream, sem, mult, op, in_bufs, out_buf, force_wait=False):
        self.counts[sem] = self.counts.get(sem, 0) + 1
        tag = (sem, self.counts[sem] * mult, stream)
        deps = []
        for b in in_bufs:
            if b.writer is not None:
                deps.append(b.writer)
        if out_buf is not None:
            deps.extend(out_buf.readers)
            if out_buf.writer is not None:
                deps.append(out_buf.writer)
        m = {}
        for dsem, dval, dstream in deps:
            if dstream == stream and not force_wait:
                continue  # same engine stream: program order
            m[dsem] = max(m.get(dsem, 0), dval)
        op["waits"] = m
        getattr(self, stream).append(op)
        for b in in_bufs:
            b.readers.append(tag)
        if out_buf is not None:
            out_buf.writer = tag
            out_buf.readers = []


def build(nslot, reps=1, with_bias=False, probe=None):
    nc = bass.Bass()
    P = nc.declare_dram_parameter

    # All input DMAs are 128-partition-wide: DMA completion semaphores fire
    # per-SDMA-engine, so a narrow (sub-128-partition) transfer's +16 can be
    # outrun by later wide transfers and cumulative waits become unsound.
    # Weights are stored host-side in chunk-major SBUF layout so each phase
    # needs only 1-2 large contiguous DMAs (the SP HWDGE queue serializes
    # DMAs, so per-DMA fixed cost is on the critical path).
    wsea = P("wsea", [nslot, 64, 2048], BF16, isOutput=False)    # wse1 | wae1
    wse2 = P("wse2", [nslot, 128, 6144], BF16, isOutput=False)   # 4 chunks
    w2b = P("w2b", [nslot, 2, 128, 4608], BF16, isOutput=False)  # 2x6 chunks
    w2a = P("w2a", [nslot, 2, 128, 4608], BF16, isOutput=False)
    w3 = P("w3", [nslot, 2, 128, 4608], BF16, isOutput=False)    # 2x3 chunks
    pin = P("pin", [nslot, 128, PIN_W], BF16, isOutput=False)
    consts = P("consts", [128, 384], BF16, isOutput=False)          # iden|onesel|ones
    biasd = (P("biasd", [nslot, 128, BIA_W], BF16, isOutput=False)
             if with_bias else None)   # row 0 used
    ao = P("ao", [nslot, 128, EMB], BF16, isOutput=True)
    st = P("st", [nslot, ITEMS_PER_SLOT, EMB], BF16, isOutput=True)

    with contextlib.ExitStack() as es:
        ec = es.enter_context
        ring = [ec(nc.sbuf_tensor(f"ring{i}", [128, 6144], BF16)) for i in range(RS)]
        ringa = [ec(nc.sbuf_tensor(f"ringa{i}", [128, 4608], BF16)) for i in range(RA)]
        pin_b = [ec(nc.sbuf_tensor(f"pin{i}", [128, PIN_W], BF16)) for i in range(2)]
        bias_b = ([ec(nc.sbuf_tensor(f"bias{i}", [128, BIA_W], BF16)) for i in range(2)]
                  if with_bias else [])
        cst_b = ec(nc.sbuf_tensor("cst_b", [128, 384], BF16))
        s_hT = ec(nc.sbuf_tensor("s_hT", [128, 16], BF16))
        s_aT = ec(nc.sbuf_tensor("s_aT", [128, EMB], BF16))
        s_tt = ec(nc.sbuf_tensor("s_tt", [ITEMS_PER_SLOT, OH], BF16))
        s_x2 = ec(nc.sbuf_tensor("s_x2", [128, OH], BF16))
        s_sg = ec(nc.sbuf_tensor("s_sg", [128, OH], BF16))
        s_x2T = ec(nc.sbuf_tensor("s_x2T", [128, OH], BF16))
        s_out = [ec(nc.sbuf_tensor(f"s_out{i}", [128, EMB], BF16)) for i in range(2)]
        s_st = [ec(nc.sbuf_tensor(f"s_st{i}", [ITEMS_PER_SLOT, EMB], BF16)) for i in range(2)]
        pA = ec(nc.psum_tensor("pA", [128, 512], F32))
        pB0 = ec(nc.psum_tensor("pB0", [128, 512], F32))
        pB1 = ec(nc.psum_tensor("pB1", [128, 512], F32))
        pC = ec(nc.psum_tensor("pC", [128, 512], F32))
        pD = ec(nc.psum_tensor("pD", [128, 512], F32))
        pE = ec(nc.psum_tensor("pE", [128, 512], F32))
        pT = ec(nc.psum_tensor("pT", [128, 512], BF16))
        s_pe = ec(nc.semaphore("s_pe"))
        s_act = ec(nc.semaphore("s_act"))
        s_dve = ec(nc.semaphore("s_dve"))
        block = ec(nc.Block())

        # ---------------- plan ----------------
        pl = _Plan()
        bufs = {
            "ring": [_Buf() for _ in range(RS)],
            "ringa": [_Buf() for _ in range(RA)],
            "pin": [_Buf() for _ in range(2)],
            "bias": [_Buf() for _ in range(2)],
            "hT": [_Buf() for _ in range(4)],
            "aT": [_Buf() for _ in range(12)],
            "tt": [_Buf() for _ in range(2)],
            "x2": [_Buf() for _ in range(2)],
            "sg": [_Buf() for _ in range(2)],
            "x2T": [_Buf() for _ in range(6)],
            "out": [_Buf() for _ in range(2)],
            "stb": [_Buf() for _ in range(2)],
            # pA/pT are single PSUM banks: PE writes and ACT/DVE reads of the
            # same bank are fatal if concurrent (P10), so track whole-tensor —
            # each new PE write waits for the previous quarter's reader.
            "pA": _Buf(),
            "pB0": _Buf(),
            "pB1": _Buf(),
            "pC": _Buf(),
            "pD": _Buf(),
            "pE": _Buf(),
            "pT": _Buf(),
            "consts": _Buf(),
        }
        rc = [0]
        rca = [0]

        def next_ringa():
            r = rca[0] % RA
            rca[0] += 1
            return r

        def dma_in(dst, dst_sl, src, src_sl, buf, key, q="sp"):
            # per-buffer DMA sems: successive writes to one buffer are ordered
            # by the WAR chain, so "sem >= 16*n" fires exactly at write n's
            # completion; a shared cumulative sem would be unsound (increments
            # from unrelated in-flight DMAs interleave).
            # q="act" issues on the ACT HWDGE ring to split queue bandwidth.
            if q == "sp":
                pl.emit("dma", "dma:" + key, 16,
                        {"dst": dst, "dst_sl": dst_sl, "src": src, "src_sl": src_sl,
                         "key": "dma:" + key},
                        [], buf)
            else:
                pl.emit("actq", "dmo:" + key, 16,
                        {"kind": "dmo", "dst": dst, "dst_sl": dst_sl, "src": src,
                         "src_sl": src_sl, "key": "dmo:" + key},
                        [], buf)

        def dma_out(dst, dst_sl, src, src_sl, buf, key):
            # on the ACT stream; force same-stream wait (DMA engines are async
            # w.r.t. the ACT pipeline, so wait for the producing copy's sem)
            pl.emit("actq", "dmo:" + key, 16,
                    {"kind": "dmo", "dst": dst, "dst_sl": dst_sl, "src": src,
                     "src_sl": src_sl, "key": "dmo:" + key}, [buf], None,
                    force_wait=True)

        def next_ring():
            r = rc[0] % RS
            rc[0] += 1
            return r

        def mm(out, out_sl, lhs, lhs_sl, rhs, rhs_sl, start, stop, in_bufs, out_buf):
            pl.emit("pe", "pe", 1,
                    {"kind": "mm", "out": out, "out_sl": out_sl, "lhs": lhs,
                     "lhs_sl": lhs_sl, "rhs": rhs, "rhs_sl": rhs_sl,
                     "start": start, "stop": stop}, in_bufs, out_buf)

        def tr(out, out_sl, in_, in_sl, in_bufs, out_buf):
            pl.emit("pe", "pe", 1,
                    {"kind": "tr", "out": out, "out_sl": out_sl, "in": in_,
                     "in_sl": in_sl}, in_bufs, out_buf)

        def act(out, out_sl, in_, in_sl, func, bias, in_bufs, out_buf):
            pl.emit("actq", "act", 1,
                    {"kind": "act", "out": out, "out_sl": out_sl, "in": in_,
                     "in_sl": in_sl, "func": func, "bias": bias}, in_bufs, out_buf)

        def dve(out, out_sl, in_, in_sl, in_bufs, out_buf):
            pl.emit("dve", "dve", 1,
                    {"out": out, "out_sl": out_sl, "in": in_, "in_sl": in_sl},
                    in_bufs, out_buf)

        # consts: one wide DMA. layout: [:,0:128]=iden, [0:4,128:256]=onesel,
        # [0:1,256:384]=ones row
        cb = bufs["consts"]
        dma_in("cst_b", np.s_[:, :], "consts", np.s_[:, :], cb, "cst")
        CS_IDEN, CS_SEL, CS_ONE = np.s_[:, 0:128], 128, 256

        def emit_slot(s, emit_prev_out):
            sb = s % 2
            pinb = bufs["pin"][sb]
            biab = bufs["bias"][sb]
            dma_in("pin_b", (sb, np.s_[:, :]), "pin", np.s_[s, :, :], pinb, f"pin{sb}")
            if with_bias:
                dma_in("bias_b", (sb, np.s_[:, :]), "biasd", np.s_[s, :, :], biab, f"bias{sb}")

            # ---- ACT-queue prefetch: second half of each big weight phase
            ra_b = next_ringa()
            dma_in("ringa", (ra_b, np.s_[:, :]), "w2b", np.s_[s, 1, :, :], bufs["ringa"][ra_b], f"ra{ra_b}", q="act")
            ra_a = next_ringa()
            dma_in("ringa", (ra_a, np.s_[:, :]), "w2a", np.s_[s, 1, :, :], bufs["ringa"][ra_a], f"ra{ra_a}", q="act")
            ra_3 = next_ringa()
            dma_in("ringa", (ra_3, np.s_[:, :]), "w3", np.s_[s, 1, :, :], bufs["ringa"][ra_3], f"ra{ra_3}", q="act")

            # ---- SE1 + (wsea mega-chunk: wse1 cols 0:512, wae1 cols 512:2048)
            r0 = next_ring()
            dma_in("ring", (r0, np.s_[0:64, 0:2048]), "wsea", np.s_[s, :, :], bufs["ring"][r0], f"r{r0}")
            for k in range(4):
                mm("pA", np.s_[0:128, k * 4:(k + 1) * 4],
                   "ring", (r0, np.s_[0:STATE_DIM, k * 128:(k + 1) * 128]),
                   "pin_b", (sb, np.s_[0:STATE_DIM, PIN_ST:PIN_ST + 4]),
                   True, True, [bufs["ring"][r0], pinb], bufs["pA"])
                act("s_hT", np.s_[:, k * 4:(k + 1) * 4], "pA", np.s_[0:128, k * 4:(k + 1) * 4],
                    AF.Relu, (sb, PIN_SB1 + k), [bufs["pA"]], bufs["hT"][k])
            # ---- SE2 (one mega-chunk: 4 k-chunks side by side) ----
            r1 = next_ring()
            dma_in("ring", (r1, np.s_[:, :]), "wse2", np.s_[s, :, :], bufs["ring"][r1], f"r{r1}")
            for k in range(4):
                for t, pn in enumerate(("pB0", "pB1", "pE")):
                    mm(pn, np.s_[0:ITEMS_PER_SLOT, 0:512],
                       "s_hT", np.s_[:, k * 4:(k + 1) * 4],
                       "ring", (r1, np.s_[:, k * 1536 + t * 512:k * 1536 + (t + 1) * 512]),
                       k == 0, (k == 3 and not with_bias),
                       [bufs["hT"][k], bufs["ring"][r1]], bufs[pn])
            if with_bias:
                for t, pn in enumerate(("pB0", "pB1", "pE")):
                    mm(pn, np.s_[0:ITEMS_PER_SLOT, 0:512],
                       "cst_b", np.s_[0:1, CS_ONE:CS_ONE + ITEMS_PER_SLOT],
                       "bias_b", (sb, np.s_[0:1, BIA_SB2 + t * 512:BIA_SB2 + (t + 1) * 512]),
                       False, True, [bufs["consts"], biab], bufs[pn])
            for t, pn in enumerate(("pB0", "pB1", "pE")):
                dve("s_st", (sb, np.s_[0:ITEMS_PER_SLOT, t * 512:(t + 1) * 512]),
                    pn, np.s_[0:ITEMS_PER_SLOT, 0:512], [bufs[pn]], bufs["stb"][sb])
            # ---- AE1 (weights already resident in r0 cols 512:2048) ----
            for j in range(12):
                q = j % 4
                mm("pA", np.s_[:, q * 128:(q + 1) * 128],
                   "ring", (r0, np.s_[0:ACT_DIM, 512 + j * 128:512 + (j + 1) * 128]),
                   "pin_b", (sb, np.s_[0:ACT_DIM, PIN_ACT:PIN_ACT + 128]),
                   True, True, [bufs["ring"][r0], pinb], bufs["pA"])
                act("s_aT", np.s_[:, j * 128:(j + 1) * 128], "pA", np.s_[:, q * 128:(q + 1) * 128],
                    AF.Identity, (sb, PIN_B1C + j), [bufs["pA"]], bufs["aT"][j])
            # ---- TT (2 mega-chunks of 6 k-chunks each) ----
            for gi in range(2):
                if gi == 0:
                    rg = next_ring()
                    dma_in("ring", (rg, np.s_[:, 0:4608]), "w2b", np.s_[s, 0, :, :], bufs["ring"][rg], f"r{rg}")
                    rn, rbuf = "ring", bufs["ring"][rg]
                else:
                    rg, rn, rbuf = ra_b, "ringa", bufs["ringa"][ra_b]
                for c in range(6):
                    k = gi * 6 + c
                    for t, pn in enumerate(("pB0", "pB1")):
                        mm(pn, np.s_[0:ITEMS_PER_SLOT, 0:384],
                           "pin_b", (sb, np.s_[0:128, PIN_TAU + k * 4:PIN_TAU + (k + 1) * 4]),
                           rn, (rg, np.s_[:, c * 768 + t * 384:c * 768 + (t + 1) * 384]),
                           k == 0, (k == 11 and not with_bias),
                           [pinb, rbuf], bufs[pn])
            if with_bias:
                for t, pn in enumerate(("pB0", "pB1")):
                    mm(pn, np.s_[0:ITEMS_PER_SLOT, 0:384],
                       "cst_b", np.s_[0:1, CS_ONE:CS_ONE + ITEMS_PER_SLOT],
                       "bias_b", (sb, np.s_[0:1, BIA_B2 + t * 384:BIA_B2 + (t + 1) * 384]),
                       False, True, [bufs["consts"], biab], bufs[pn])
            for t, pn in enumerate(("pB0", "pB1")):
                act("s_tt", np.s_[0:ITEMS_PER_SLOT, t * 384:(t + 1) * 384],
                    pn, np.s_[0:ITEMS_PER_SLOT, 0:384], AF.Copy, None,
                    [bufs[pn]], bufs["tt"][t])

            # previous slot's output DMAs, ~20 weight chunks into this slot
            emit_prev_out()

            # ---- X2 (2 mega-chunks) ----
            for gi in range(2):
                if gi == 0:
                    rg = next_ring()
                    dma_in("ring", (rg, np.s_[:, 0:4608]), "w2a", np.s_[s, 0, :, :], bufs["ring"][rg], f"r{rg}")
                    rn, rbuf = "ring", bufs["ring"][rg]
                else:
                    rg, rn, rbuf = ra_a, "ringa", bufs["ringa"][ra_a]
                for c in range(6):
                    k = gi * 6 + c
                    for t, pn in enumerate(("pC", "pD")):
                        mm(pn, np.s_[:, 0:384], "s_aT", np.s_[:, k * 128:(k + 1) * 128],
                           rn, (rg, np.s_[:, c * 768 + t * 384:c * 768 + (t + 1) * 384]),
                           k == 0, False, [bufs["aT"][k], rbuf], bufs[pn])
            for t, pn in enumerate(("pC", "pD")):
                mm(pn, np.s_[:, 0:384],
                   "cst_b", np.s_[0:ITEMS_PER_SLOT, CS_SEL:CS_SEL + 128],
                   "s_tt", np.s_[0:ITEMS_PER_SLOT, t * 384:(t + 1) * 384],
                   False, True, [bufs["consts"], bufs["tt"][t]], bufs[pn])
            for t, pn in enumerate(("pC", "pD")):
                # swish = x * sigmoid(x): ACT computes sigmoid, DVE multiplies
                act("s_sg", np.s_[:, t * 384:(t + 1) * 384], pn, np.s_[:, 0:384],
                    AF.Sigmoid, None, [bufs[pn]], bufs["sg"][t])
                pl.emit("dve", "dve", 1,
                        {"kind": "mul",
                         "out": "s_x2", "out_sl": np.s_[:, t * 384:(t + 1) * 384],
                         "in": pn, "in_sl": np.s_[:, 0:384],
                         "in2": "s_sg", "in2_sl": np.s_[:, t * 384:(t + 1) * 384]},
                        [bufs[pn], bufs["sg"][t]], bufs["x2"][t])
            # ---- TR ----
            for t in range(6):
                q = t % 4
                tr("pT", np.s_[:, q * 128:(q + 1) * 128], "s_x2", np.s_[:, t * 128:(t + 1) * 128],
                   [bufs["x2"][t // 3]], bufs["pT"])
                dve("s_x2T", np.s_[:, t * 128:(t + 1) * 128], "pT", np.s_[:, q * 128:(q + 1) * 128],
                    [bufs["pT"]], bufs["x2T"][t])
            # ---- AE3 (2 mega-chunks of 3 k-chunks) ----
            for gi in range(2):
                if gi == 0:
                    rg = next_ring()
                    dma_in("ring", (rg, np.s_[:, 0:4608]), "w3", np.s_[s, 0, :, :], bufs["ring"][rg], f"r{rg}")
                    rn, rbuf = "ring", bufs["ring"][rg]
                else:
                    rg, rn, rbuf = ra_3, "ringa", bufs["ringa"][ra_3]
                for c in range(3):
                    k = gi * 3 + c
                    for t, pn in enumerate(("pC", "pD", "pE")):
                        mm(pn, np.s_[:, 0:512], "s_x2T", np.s_[:, k * 128:(k + 1) * 128],
                           rn, (rg, np.s_[:, c * 1536 + t * 512:c * 1536 + (t + 1) * 512]),
                           k == 0, (k == 5 and not with_bias),
                           [bufs["x2T"][k], rbuf], bufs[pn])
            if with_bias:
                for t, pn in enumerate(("pC", "pD", "pE")):
                    mm(pn, np.s_[:, 0:512],
                       "cst_b", np.s_[0:1, CS_ONE:CS_ONE + 128],
                       "bias_b", (sb, np.s_[0:1, BIA_B3 + t * 512:BIA_B3 + (t + 1) * 512]),
                       False, True, [bufs["consts"], biab], bufs[pn])
            for t, pn in enumerate(("pC", "pD", "pE")):
                dve("s_out", (sb, np.s_[:, t * 512:(t + 1) * 512]), pn, np.s_[:, 0:512],
                    [bufs[pn]], bufs["out"][sb])

        def make_out_emitter(s):
            def f():
                sb = s % 2
                dma_out("ao", np.s_[s, :, :], "s_out", (sb, np.s_[:, :]), bufs["out"][sb], f"out{sb}")
                dma_out("st", np.s_[s, :, :], "s_st", (sb, np.s_[:, :]), bufs["stb"][sb], f"st{sb}")
            return f

        pending = lambda: None  # noqa: E731
        for rep in range(reps):
            for s in range(nslot):
                emit_slot(s, pending)
                pending = make_out_emitter(s)
        pending()

        # ---------------- emit ----------------
        dma_sems = {k: ec(nc.semaphore("sem_" + k.replace(":", "_")))
                    for k in pl.counts if k.startswith(("dma:", "dmo:"))}

        tensors = {
            "ring": ring, "ringa": ringa, "pin_b": pin_b, "bias_b": bias_b, "cst_b": cst_b,
            "s_hT": s_hT, "s_aT": s_aT,
            "s_tt": s_tt, "s_x2": s_x2, "s_sg": s_sg, "s_x2T": s_x2T, "s_out": s_out, "s_st": s_st,
            "pA": pA, "pB0": pB0, "pB1": pB1, "pC": pC, "pD": pD, "pE": pE, "pT": pT,
            "wsea": wsea, "wse2": wse2, "w2b": w2b, "w2a": w2a,
            "w3": w3, "pin": pin, "biasd": biasd, "consts": consts,
            "ao": ao, "st": st,
        }

        def ap(name, sl):
            t = tensors[name]
            if isinstance(t, list):
                i, s2 = sl
                return t[i][s2]
            return t[sl]

        sems = {"pe": s_pe, "act": s_act, "dve": s_dve}

        def make_waiter(eng_handle):
            hw = {}

            def wait(wmap):
                for sname in sorted(wmap):
                    val = wmap[sname]
                    if hw.get(sname, 0) >= val:
                        continue
                    hw[sname] = val
                    h = sems[sname] if sname in sems else dma_sems[sname]
                    eng_handle.wait_ge(h, val)

            return wait

        if probe == "pe":
            pl.dma = []
        if probe in ("dma", "pe"):
            for _lst in (pl.dma, pl.pe, pl.actq, pl.dve):
                for _op in _lst:
                    _op["waits"] = {}
        if probe == "dma":
            # self-throttle: each DMA waits for the previous write to its own
            # buffer (ring depth flow control without compute)
            _kc = {}
            for _op in pl.dma:
                _k = _op["key"]
                if _kc.get(_k, 0) > 0:
                    _op["waits"] = {_k: 16 * _kc[_k]}
                _kc[_k] = _kc.get(_k, 0) + 1
        if probe == "dma":
            pl.pe = []
            pl.actq = [o for o in pl.actq if o["kind"] != "act"]
            # tiny DVE read per input DMA so walrus keeps the transfers live
            pl.dve = [{"out": "s_hT", "out_sl": np.s_[0:4, 0:4],
                       "in": op["dst"],
                       "in_sl": (op["dst_sl"] if not isinstance(op["dst_sl"], tuple)
                                 or not isinstance(op["dst_sl"][0], int)
                                 else op["dst_sl"]),
                       "probe_read": True, "waits": {}}
                      for op in pl.dma]
            for op in pl.dve:
                sl = op["in_sl"]
                if isinstance(sl, tuple) and isinstance(sl[0], int):
                    op["in_sl"] = (sl[0], np.s_[0:4, 0:4])
                else:
                    op["in_sl"] = np.s_[0:4, 0:4]
        if probe == "pe":
            pl.actq = []
            pl.dve = []

        @block.sync
        def _(sync):
            wait = make_waiter(sync)
            cnt = {}
            for op in pl.dma:
                wait(op["waits"])
                k = op["key"]
                cnt[k] = cnt.get(k, 0) + 16
                sync.dma_start(out=ap(op["dst"], op["dst_sl"]),
                               in_=ap(op["src"], op["src_sl"])).then_inc(dma_sems[k], 16)
            for k, v in sorted(cnt.items()):
                sync.wait_ge(dma_sems[k], v)

        @block.tensor
        def _(pe):
            wait = make_waiter(pe)
            for op in pl.pe:
                wait(op["waits"])
                if op["kind"] == "mm":
                    pe.matmul(ap(op["out"], op["out_sl"]), ap(op["lhs"], op["lhs_sl"]),
                              ap(op["rhs"], op["rhs_sl"]), start=op["start"],
                              stop=op["stop"]).then_inc(s_pe, 1)
                else:
                    pe.transpose(ap(op["out"], op["out_sl"]), ap(op["in"], op["in_sl"]),
                                 cst_b[:, 0:128]).then_inc(s_pe, 1)

        @block.scalar
        def _(a):
            wait = make_waiter(a)
            dmo_cnt = {}
            for op in pl.actq:
                wait(op["waits"])
                if op["kind"] == "dmo":
                    k = op["key"]
                    dmo_cnt[k] = dmo_cnt.get(k, 0) + 16
                    a.dma_start(out=ap(op["dst"], op["dst_sl"]),
                                in_=ap(op["src"], op["src_sl"])).then_inc(dma_sems[k], 16)
                elif op["bias"] is None:
                    a.activation(ap(op["out"], op["out_sl"]), ap(op["in"], op["in_sl"]),
                                 op["func"]).then_inc(s_act, 1)
                else:
                    bi, bc = op["bias"]
                    bias_ap = pin_b[bi][:, bc:bc + 1]
                    a.activation(ap(op["out"], op["out_sl"]), ap(op["in"], op["in_sl"]),
                                 op["func"], bias=bias_ap).then_inc(s_act, 1)
            for k, v in sorted(dmo_cnt.items()):
                a.wait_ge(dma_sems[k], v)

        @block.vector
        def _(v):
            wait = make_waiter(v)
            for op in pl.dve:
                wait(op["waits"])
                if op.get("kind") == "mul":
                    v.tensor_mul(ap(op["out"], op["out_sl"]),
                                 ap(op["in"], op["in_sl"]),
                                 ap(op["in2"], op["in2_sl"])).then_inc(s_dve, 1)
                else:
                    v.tensor_copy(ap(op["out"], op["out_sl"]),
                                  ap(op["in"], op["in_sl"])).then_inc(s_dve, 1)

    return nc


# ---------------------------------------------------------------------------
# Host-side routing, gathering, execution, unsharding
# ---------------------------------------------------------------------------
def plan_units(cat_ids):
    """Return list of units (cat, items(<=4), half) in a deterministic order."""
    order = {}
    for b, g in enumerate(cat_ids.tolist()):
        order.setdefault(g, []).append(b)
    units = []
    for g in sorted(order):
        items = order[g]
        for i0 in range(0, len(items), ITEMS_PER_SLOT):
            grp = items[i0:i0 + ITEMS_PER_SLOT]
            for h in range(2):
                units.append((g, grp, h))
    return units


def make_inputs(units_core, nslot, state, actions, tau_np,
                se_W1, se_b1, se_W2, se_b2,
                ae_W1, ae_b1, ae_W2, ae_b2, ae_W3, ae_b3, with_bias=None):
    if with_bias is None:
        with_bias = any(np.any(a) for a in (ae_b2, ae_b3, se_b2))
    z = np.zeros
    f = np.float32
    consts = z((128, 384), f)
    consts[:, 0:128] = np.eye(128, dtype=f)
    consts[0:ITEMS_PER_SLOT, 128:256] = np.kron(np.eye(ITEMS_PER_SLOT, dtype=f),
                                                np.ones((1, T), f))
    consts[0, 256:384] = 1.0
    d = {
        "wsea": z((nslot, 64, 2048), f),
        "wse2": z((nslot, 128, 6144), f),
        "w2b": z((nslot, 2, 128, 4608), f),
        "w2a": z((nslot, 2, 128, 4608), f),
        "w3": z((nslot, 2, 128, 4608), f),
        "pin": z((nslot, 128, PIN_W), f),
        "consts": consts,
    }

    def chunk_major(w, groups, chunks, width):
        # [groups*chunks*128, width] -> [groups, 128, chunks*width]
        return (w.reshape(groups, chunks, 128, width)
                .transpose(0, 2, 1, 3).reshape(groups, 128, chunks * width))
    if with_bias:
        d["biasd"] = z((nslot, 128, BIA_W), f)
    for s, (g, items, h) in enumerate(units_core):
        H = slice(h * HH, (h + 1) * HH)
        O = slice(h * OH, (h + 1) * OH)
        d["wsea"][s][:STATE_DIM, 0:HH] = se_W1[g][:, H]
        d["wsea"][s][:ACT_DIM, HH:HH + EMB] = ae_W1[g]
        d["wse2"][s] = chunk_major(se_W2[g][H, :], 1, 4, EMB)[0]
        d["w2b"][s] = chunk_major(ae_W2[g][EMB:, O], 2, 6, OH)
        d["w2a"][s] = chunk_major(ae_W2[g][:EMB, O], 2, 6, OH)
        d["w3"][s] = chunk_major(ae_W3[g][O, :], 2, 3, EMB)
        p = d["pin"][s]
        p[:, PIN_B1C:PIN_B1C + 12] = ae_b1[g].reshape(12, 128).T
        p[:, PIN_SB1:PIN_SB1 + 4] = se_b1[g][H].reshape(4, 128).T
        tau3 = p[:, PIN_TAU:PIN_TAU + 48].reshape(128, 12, ITEMS_PER_SLOT)
        for i, b in enumerate(items):
            tau3[:, :, i] = tau_np[b].reshape(12, 128).T
            p[0:ACT_DIM, PIN_ACT + i * T:PIN_ACT + (i + 1) * T] = actions[b].T
            p[0:STATE_DIM, PIN_ST + i] = state[b, 0]
        if with_bias:
            bb = d["biasd"][s][0]
            bb[BIA_B2:BIA_B2 + OH] = ae_b2[g][O]
            if h == 0:
                bb[BIA_B3:BIA_B3 + EMB] = ae_b3[g]
                bb[BIA_SB2:BIA_SB2 + EMB] = se_b2[g]
    return {k: v.astype(NPBF16) for k, v in d.items()}


def kernel(state, actions, timesteps, cat_ids,
           se_W1, se_b1, se_W2, se_b2,
           ae_W1, ae_b1, ae_W2, ae_b2, ae_W3, ae_b3):
    args = [np.asarray(a) for a in (state, actions, timesteps, cat_ids, se_W1, se_b1,
                                    se_W2, se_b2, ae_W1, ae_b1, ae_W2, ae_b2, ae_W3, ae_b3)]
    (state, actions, timesteps, cat_ids, se_W1, se_b1, se_W2, se_b2,
     ae_W1, ae_b1, ae_W2, ae_b2, ae_W3, ae_b3) = args
    tau_np = _sinusoid(timesteps)

    units = plan_units(cat_ids)
    nslot = max(1, -(-len(units) // N_CORES))
    per_core = [[] for _ in range(N_CORES)]
    for i, u in enumerate(units):
        per_core[i % N_CORES].append(u)
    for c in range(N_CORES):
        while len(per_core[c]) < nslot:
            per_core[c].append(None)  # dummy

    with_bias = bool(any(np.any(a) for a in (ae_b2, ae_b3, se_b2)))
    in_maps = []
    for c in range(N_CORES):
        units_c = [(u if u is not None else units[0]) for u in per_core[c]]
        in_maps.append(make_inputs(units_c, nslot, state, actions, tau_np,
                                   se_W1, se_b1, se_W2, se_b2,
                                   ae_W1, ae_b1, ae_W2, ae_b2, ae_W3, ae_b3,
                                   with_bias=with_bias))

    nc = build(nslot, with_bias=with_bias)
    res = run_bass_kernel_spmd(nc, in_maps, list(range(N_CORES)))

    out = np.zeros((B, T + 1, EMB), np.float32)
    for c in range(N_CORES):
        ao = res.results[c]["ao"].astype(np.float32)
        stx = res.results[c]["st"].astype(np.float32)
        for s, u in enumerate(per_core[c]):
            if u is None:
                continue
            g, items, h = u
            for i, b in enumerate(items):
                out[b, 0] += stx[s, i]
                out[b, 1:] += ao[s, i * T:(i + 1) * T]
    return out
